# revision 26
# baseline (speedup 1.0000x reference)
"""Trainium2 Bass kernel for nn_Attribution (sparse local-window attention).

Data-parallel over batch n=8 -> one batch element per NeuronCore.

Per-core computation (c_in=256, ch=128, 64x64 image):
    h    = W1 @ x + b1
    corr = 5x5 local window correlation of h (zero padded), /sqrt(128)
    attn = softmax over the 25 window entries
    samp = sum_k attn_k * shift_k(h)
    gate = sigmoid(relu(W2 @ h + b2))
    out  = Wout @ (gate * samp) + bout

Layout: positions flattened row-major with 2 zero-pad rows top/bottom
(68 rows x 64 = 4352 positions = 34 chunks of 128).  Scores are "born
transposed" (keys of chunk c on psum partitions, queries on free axis).
exp'd+masked scores live c-major in attnm: chunk c block a (query sub
s=c-2+a) at cols 384c+128a.

Key structure vs a naive implementation:
  - hT (position-major h) via 4 DMA-transpose instructions, not PE.
  - denominators are computed REPLICATED across partitions by using an
    all-2.0 (128,128) stationary, so softmax normalization is a single
    DVE divide; the zero-pad correction 2*D is preloaded into the psum
    accumulator by a K=1 matmul.  Factor 2 matches Pg = 2*gate =
    1+relu(tanh(z/2+b2/2)).
  - out-conv bias: oc0 added by DVE during psum evac; oc1 preloaded
    into psum (K=1 matmul) and evac'd by ACT copy.  Output is written
    bf16 and widened to f32 on host.
  - per-chunk pipeline with LAG so PE never waits on ACT exp / DVE
    mask: [score c] ... [sample c-3, den c-3] interleaved.
"""
import sys

sys.path.insert(0, "/opt/trn_rl_repo")

import numpy as np
import ml_dtypes

import concourse.bass as bass
import concourse.mybir as mybir
import concourse.tile as tile
from concourse import bacc
from concourse.bass_utils import run_bass_kernel_spmd

F32 = mybir.dt.float32
BF16 = mybir.dt.bfloat16
AF = mybir.ActivationFunctionType
ALU = mybir.AluOpType

N, CIN, CH, H, W = 8, 256, 128, 64, 64
HW = H * W                      # 4096
RAD = 2
KROWS = H + 2 * RAD             # 68 padded rows
PADPOS = KROWS * W              # 4352
NCHUNK = PADPOS // 128          # 34 key chunks (2 rows each)
NSUB = H // 2                   # 32 query subs (128 queries each)
SCALE = 1.0 / np.sqrt(np.float32(CH))
LAG = 5

# CB (bf16 const block) column layout
CB_W1T0 = 0
CB_W1T1 = 128
CB_W2T = 256
CB_WOT = 384          # (128, 256)
CB_MASK = 640         # (128, 384)
CB_TWOS = 1024        # (128, 128) of 2.0
CB_D512 = 1152        # row 0: (1, 512) pad-correction D
CB_BOUT1 = 1664       # row 0: (1, 128) bout[128:256]
CB_ONES512 = 1792     # row 0: (1, 512) of 1.0
CB_COLS = 2304


def _build_mask_and_D():
    """maskC: (128, 384) {0,1}; col 128*a+q is the score of key (chunk c,
    pos p) vs query q of sub s = c-2+a.  Valid iff |2-2a + p//64 - q//64|
    <= 2 and |p%64 - q%64| <= 2.   D: (512,) = 5*cnt(qx) tiled (the number
    of window slots per query that fall off the row ends; each contributes
    exp(0)=1 to the reference softmax denominator)."""
    m = np.zeros((128, 384), dtype=np.float32)
    for a in range(3):
        for p in range(128):
            for q in range(128):
                dy = 2 - 2 * a + p // 64 - q // 64
                if abs(dy) <= RAD and abs(p % 64 - q % 64) <= RAD:
                    m[p, 128 * a + q] = 1.0
    cnt = np.array([sum(1 for dx in range(-RAD, RAD + 1) if not 0 <= qx + dx < W)
                    for qx in range(W)], dtype=np.float32)
    Drow = 5.0 * np.concatenate([cnt, cnt, cnt, cnt])   # (256,) -> tile to 512
    D512 = np.concatenate([Drow, Drow])[:512]
    return m, D512


def build_nc(repeat=1, sim_safe=False, dbg=False):
    nc = bacc.Bacc("TRN2", target_bir_lowering=False, debug=False, num_devices=8)

    x_d = nc.declare_dram_parameter("x", [CIN, HW], BF16, isOutput=False)
    cb_d = nc.declare_dram_parameter("CB", [128, CB_COLS], BF16, isOutput=False)
    bf_d = nc.declare_dram_parameter("BF", [128, 4], F32, isOutput=False)
    out_d = nc.declare_dram_parameter("out", [CIN, HW], BF16, isOutput=True)
    if dbg:
        dbg_d = {nm: nc.declare_dram_parameter(f"dbg_{nm}", shp, dt, isOutput=True)
                 for nm, shp, dt in [
                     ("hpad", [128, PADPOS], BF16), ("hT", [128, PADPOS], BF16),
                     ("attnm", [128, NCHUNK * 384], BF16), ("Pg", [128, HW], BF16),
                     ("rden", [128, HW], F32), ("spg", [128, HW], BF16)]}

    with tile.TileContext(nc) as tc:
        with (
            tc.tile_pool(name="per", bufs=1) as per,
            tc.tile_pool(name="psc", bufs=3, space="PSUM") as psc,   # scores
            tc.tile_pool(name="psa", bufs=2, space="PSUM") as psa,   # conv2 / sample
            tc.tile_pool(name="pdn", bufs=2, space="PSUM") as pdn,   # conv1 / den
            tc.tile_pool(name="pou", bufs=1, space="PSUM") as pou,   # convout
        ):
            xsb0 = per.tile([128, HW], BF16, tag="xsb0")
            xsb1 = per.tile([128, HW], BF16, tag="xsb1")
            hpad = per.tile([128, PADPOS], BF16, tag="hpad")
            hT = per.tile([128, PADPOS], BF16, tag="hT")
            attnm = per.tile([128, NCHUNK * 384], BF16, tag="attnm")
            Pg = per.tile([128, HW], BF16, tag="Pg")
            spg = per.tile([128, HW], BF16, tag="spg")
            outsb = per.tile([128, 2 * HW], BF16, tag="outsb")
            rden = per.tile([128, HW], F32, tag="rden")
            cb = per.tile([128, CB_COLS], BF16, tag="cb")
            bfc = per.tile([128, 4], F32, tag="bfc")

            for _rep in range(repeat):
                # ---- input + const DMAs (4KB-contiguous runs, three queues;
                # the two col-half-0 pieces go to different queues so conv1
                # t0-3 isn't gated on a serialized queue)
                nc.sync.dma_start(xsb0[:, 0:2048], x_d[0:128, 0:2048])
                nc.scalar.dma_start(cb[:], cb_d[:])
                nc.scalar.dma_start(bfc[:], bf_d[:])
                nc.scalar.dma_start(xsb1[:, 0:2048], x_d[128:256, 0:2048])
                nc.sync.dma_start(xsb0[:, 2048:4096], x_d[0:128, 2048:4096])
                nc.gpsimd.dma_start(xsb1[:, 2048:4096], x_d[128:256, 2048:4096])
                nc.gpsimd.memset(hpad[:, 0:128], 0.0)
                nc.gpsimd.memset(hpad[:, PADPOS - 128:PADPOS], 0.0)

                # ---- PE warmup: ~4us of dummy matmuls during the x-DMA wait
                # ramps the PE p-state (full clock needs ~3us continuous) so
                # conv1 runs at 2.4GHz instead of cold-clock
                warm = pou.tile([128, 512], F32, tag="po", name="warm")
                for _w in range(10):
                    nc.tensor.matmul(warm[:], cb[:, 0:128], cb[:, 0:512],
                                     start=True, stop=True)

                # ---- P1: conv1 (+bias on DVE), conv2 gate lagging 2 tiles
                def emit_conv1(t):
                    pc = pdn.tile([128, 512], F32, tag="pc")
                    sl = slice(512 * t, 512 * (t + 1))
                    nc.tensor.matmul(pc[:], cb[:, CB_W1T0:CB_W1T0 + 128],
                                     xsb0[:, sl], start=True, stop=False)
                    nc.tensor.matmul(pc[:], cb[:, CB_W1T1:CB_W1T1 + 128],
                                     xsb1[:, sl], start=False, stop=True)
                    nc.vector.tensor_scalar(
                        out=hpad[:, 128 + 512 * t:128 + 512 * (t + 1)], in0=pc[:],
                        scalar1=bfc[:, 0:1], scalar2=None, op0=ALU.add)

                def emit_conv2(t):
                    pz = psa.tile([128, 512], F32, tag="pz")
                    hsl = slice(128 + 512 * t, 128 + 512 * (t + 1))
                    sl = slice(512 * t, 512 * (t + 1))
                    nc.tensor.matmul(pz[:], cb[:, CB_W2T:CB_W2T + 128],
                                     hpad[:, hsl], start=True, stop=True)
                    # Pg = 1 + relu(tanh(z/2 + b2/2)) = 2*sigmoid(relu(z))
                    nc.scalar.activation(Pg[:, sl], pz[:], AF.Tanh,
                                         scale=0.5, bias=bfc[:, 1:2])
                    nc.vector.tensor_scalar(out=Pg[:, sl], in0=Pg[:, sl],
                                            scalar1=0.0, scalar2=1.0,
                                            op0=ALU.max, op1=ALU.add)

                for t in range(8):
                    emit_conv1(t)
                    if t >= 2:
                        emit_conv2(t - 2)
                emit_conv2(6)
                emit_conv2(7)

                # ---- hT via DMA transpose (4 quarters of 9/9/9/7 chunks)
                for q in range(4):
                    c0, c1 = 9 * q, min(NCHUNK, 9 * (q + 1))
                    cols = slice(128 * c0, 128 * c1)
                    nc.sync.dma_start(
                        hT[:, cols].rearrange("p (c k) -> p c k", k=128),
                        hpad[:, cols], transpose=True)

                # ---- P2: per-chunk pipeline
                sc_meta = {}
                samp_tiles = {}
                den_tiles = {}
                po_tiles = {}

                def emit_score(c):
                    lo, hi = max(0, c - 2), min(NSUB - 1, c)
                    alo, ahi = lo - (c - 2), hi - (c - 2)
                    sc = psc.tile([128, 512], F32, tag="sc")
                    sc_meta[c] = (alo, ahi)
                    psl = slice(128 * alo, 128 * (ahi + 1))
                    nc.tensor.matmul(sc[:, psl],
                                     hpad[:, 128 * c:128 * (c + 1)],
                                     hpad[:, 128 * (lo + 1):128 * (hi + 2)],
                                     start=True, stop=True)
                    asl = attnm[:, 384 * c + 128 * alo:384 * c + 128 * (ahi + 1)]
                    nc.scalar.activation(asl, sc[:, psl], AF.Exp, scale=float(SCALE))
                    meng = nc.vector if c % 3 == 0 else nc.gpsimd
                    meng.tensor_tensor(
                        out=asl, in0=asl,
                        in1=cb[:, CB_MASK + 128 * alo:CB_MASK + 128 * (ahi + 1)],
                        op=ALU.mult)

                def emit_sampden(c):
                    alo, ahi = sc_meta[c]
                    if c % 4 == 0 and c < 32:
                        # tiles for supersub G=c//4: first write is chunk 4G.
                        # Each bank gets exactly ONE start=True (the K=1
                        # preload); all later MMs accumulate, so no open
                        # accumulation group is ever wiped.
                        G = c // 4
                        den_tiles[G] = pdn.tile([128, 512], F32, tag="pc",
                                                name=f"dn{G}")
                        nc.tensor.matmul(den_tiles[G][:],
                                         cb[0:1, CB_TWOS:CB_TWOS + 128],
                                         cb[0:1, CB_D512:CB_D512 + 512],
                                         start=True, stop=False,
                                         skip_group_check=True)
                        samp_tiles[G] = psa.tile([128, 512], F32, tag="pz",
                                                 name=f"sp{G}")
                        nc.tensor.matmul(samp_tiles[G][:],
                                         cb[32:33, CB_D512:CB_D512 + 128],
                                         cb[32:33, CB_D512:CB_D512 + 512],
                                         start=True, stop=False,
                                         skip_group_check=True)
                    # sample + den: one MM per contiguous run of subs in the
                    # same psum tile (chunk c's 3 blocks share lhsT hT[c] and
                    # are contiguous in attnm)
                    a = alo
                    while a <= ahi:
                        s = c - 2 + a
                        G, j = s // 4, s % 4
                        na = min(ahi - a + 1, 4 - j)
                        blk = attnm[:, 384 * c + 128 * a:384 * c + 128 * (a + na)]
                        nc.tensor.matmul(
                            samp_tiles[G][:, 128 * j:128 * (j + na)],
                            hT[:, 128 * c:128 * (c + 1)], blk,
                            start=False, stop=(alo == 0), skip_group_check=True)
                        nc.tensor.matmul(
                            den_tiles[G][:, 128 * j:128 * (j + na)],
                            cb[:, CB_TWOS:CB_TWOS + 128], blk,
                            start=False, stop=(alo == 0), skip_group_check=True)
                        a += na

                def emit_finish1(G):
                    # softmax-normalize + gate on DVE (2 chunks before the
                    # convout MMs enter the PE queue, so the PE never stalls
                    # on this chain)
                    qsl = slice(512 * G, 512 * (G + 1))
                    sp_ = samp_tiles.pop(G)
                    den_ = den_tiles.pop(G)
                    nc.vector.reciprocal_approx_fast(out=rden[:, qsl], in_=den_[:])
                    nc.vector.tensor_tensor(out=spg[:, qsl], in0=sp_[:],
                                            in1=Pg[:, qsl], op=ALU.mult)
                    nc.vector.tensor_tensor(out=spg[:, qsl], in0=spg[:, qsl],
                                            in1=rden[:, qsl], op=ALU.mult)

                def emit_finish2(G):
                    qsl = slice(512 * G, 512 * (G + 1))
                    po = pou.tile([128, 512], F32, tag="po", name=f"po{G}a")
                    po_tiles[G] = po
                    nc.tensor.matmul(po[:], cb[:, CB_WOT:CB_WOT + 128],
                                     spg[:, qsl], start=True, stop=True)
                    nc.vector.tensor_scalar(
                        out=outsb[:, qsl], in0=po[:],
                        scalar1=bfc[:, 2:3], scalar2=None, op0=ALU.add)
                    if G % 2 == 1:
                        # oc0 rows for supersub pair (G-1, G): 2KB runs
                        psl = slice(512 * (G - 1), 512 * (G + 1))
                        nc.sync.dma_start(out_d[0:128, psl], outsb[:, psl])

                def emit_finish3(G):
                    qsl = slice(512 * G, 512 * (G + 1))
                    po_tiles.pop(G)
                    po2 = pou.tile([128, 512], F32, tag="po", name=f"po{G}b")
                    nc.tensor.matmul(po2[:], cb[0:1, CB_BOUT1:CB_BOUT1 + 128],
                                     cb[0:1, CB_ONES512:CB_ONES512 + 512],
                                     start=True, stop=False, skip_group_check=True)
                    nc.tensor.matmul(po2[:], cb[:, CB_WOT + 128:CB_WOT + 256],
                                     spg[:, qsl], start=False, stop=True,
                                     skip_group_check=True)
                    nc.scalar.activation(outsb[:, 4096 + 512 * G:4096 + 512 * (G + 1)],
                                         po2[:], AF.Copy)
                    if G % 2 == 1:
                        psl = slice(512 * (G - 1), 512 * (G + 1))
                        nc.scalar.dma_start(out_d[128:256, psl],
                                            outsb[:, 4096 + psl.start:4096 + psl.stop])

                for cc in range(NCHUNK + LAG + 4):
                    if cc < NCHUNK:
                        emit_score(cc)
                    d = cc - LAG
                    if 0 <= d < NCHUNK:
                        emit_sampden(d)
                    if d >= 5 and (d - 5) % 4 == 0 and (d - 5) // 4 < 8:
                        emit_finish1((d - 5) // 4)
                    if d >= 7 and (d - 7) % 4 == 0 and (d - 7) // 4 < 8:
                        emit_finish2((d - 7) // 4)
                    if d >= 8 and (d - 8) % 4 == 0 and (d - 8) // 4 < 8:
                        emit_finish3((d - 8) // 4)

                if dbg:
                    for nm, t in [("hpad", hpad), ("hT", hT), ("attnm", attnm),
                                  ("Pg", Pg), ("rden", rden), ("spg", spg)]:
                        nc.sync.dma_start(dbg_d[nm][:], t[:])

    return nc


def _prep_inputs(x, W1, b1, W2, b2, Wout, bout):
    maskC, D512 = _build_mask_and_D()
    bf = ml_dtypes.bfloat16
    CB = np.zeros((128, CB_COLS), np.float32)
    W1T = np.ascontiguousarray(np.asarray(W1, np.float32).T)   # (256, 128)
    CB[:, CB_W1T0:CB_W1T0 + 128] = W1T[0:128]
    CB[:, CB_W1T1:CB_W1T1 + 128] = W1T[128:256]
    CB[:, CB_W2T:CB_W2T + 128] = np.asarray(W2, np.float32).T
    CB[:, CB_WOT:CB_WOT + 256] = np.asarray(Wout, np.float32).T
    CB[:, CB_MASK:CB_MASK + 384] = maskC
    CB[:, CB_TWOS:CB_TWOS + 128] = 2.0
    CB[0, CB_D512:CB_D512 + 512] = D512
    CB[0, CB_BOUT1:CB_BOUT1 + 128] = np.asarray(bout, np.float32)[128:256]
    CB[0, CB_ONES512:CB_ONES512 + 512] = 1.0
    BF = np.zeros((128, 4), np.float32)
    BF[:, 0] = np.asarray(b1, np.float32)
    BF[:, 1] = 0.5 * np.asarray(b2, np.float32)
    BF[:, 2] = np.asarray(bout, np.float32)[0:128]
    common = {"CB": CB.astype(bf), "BF": BF}
    in_maps = []
    for i in range(N):
        m = dict(common)
        m["x"] = np.ascontiguousarray(
            np.asarray(x[i], np.float32).reshape(CIN, HW)).astype(bf)
        in_maps.append(m)
    return in_maps


_CACHED = {}


def kernel(x, W1, b1, W2, b2, Wout, bout):
    if "nc" not in _CACHED:
        nc = build_nc()
        nc.finalize()
        _CACHED["nc"] = nc
    nc = _CACHED["nc"]
    in_maps = _prep_inputs(x, W1, b1, W2, b2, Wout, bout)
    res = run_bass_kernel_spmd(nc, in_maps, core_ids=list(range(N)))
    out = np.stack([np.asarray(res.results[i]["out"], dtype=np.float32)
                    .reshape(CIN, H, W) for i in range(N)])
    return out


# revision 27
# speedup vs baseline: 1.0135x; 1.0135x over previous
"""Trainium2 Bass kernel for nn_Attribution (sparse local-window attention).

Data-parallel over batch n=8 -> one batch element per NeuronCore.

Per-core computation (c_in=256, ch=128, 64x64 image):
    h    = W1 @ x + b1
    corr = 5x5 local window correlation of h (zero padded), /sqrt(128)
    attn = softmax over the 25 window entries
    samp = sum_k attn_k * shift_k(h)
    gate = sigmoid(relu(W2 @ h + b2))
    out  = Wout @ (gate * samp) + bout

Layout: positions flattened row-major with 2 zero-pad rows top/bottom
(68 rows x 64 = 4352 positions = 34 chunks of 128).  Scores are "born
transposed" (keys of chunk c on psum partitions, queries on free axis).
exp'd+masked scores live c-major in attnm: chunk c block a (query sub
s=c-2+a) at cols 384c+128a.

Key structure vs a naive implementation:
  - hT (position-major h) via 4 DMA-transpose instructions, not PE.
  - denominators are computed REPLICATED across partitions by using an
    all-2.0 (128,128) stationary, so softmax normalization is a single
    DVE divide; the zero-pad correction 2*D is preloaded into the psum
    accumulator by a K=1 matmul.  Factor 2 matches Pg = 2*gate =
    1+relu(tanh(z/2+b2/2)).
  - out-conv bias: oc0 added by DVE during psum evac; oc1 preloaded
    into psum (K=1 matmul) and evac'd by ACT copy.  Output is written
    bf16 and widened to f32 on host.
  - per-chunk pipeline with LAG so PE never waits on ACT exp / DVE
    mask: [score c] ... [sample c-3, den c-3] interleaved.
"""
import sys

sys.path.insert(0, "/opt/trn_rl_repo")

import numpy as np
import ml_dtypes

import concourse.bass as bass
import concourse.mybir as mybir
import concourse.tile as tile
from concourse import bacc
from concourse.bass_utils import run_bass_kernel_spmd

F32 = mybir.dt.float32
BF16 = mybir.dt.bfloat16
AF = mybir.ActivationFunctionType
ALU = mybir.AluOpType

N, CIN, CH, H, W = 8, 256, 128, 64, 64
HW = H * W                      # 4096
RAD = 2
KROWS = H + 2 * RAD             # 68 padded rows
PADPOS = KROWS * W              # 4352
NCHUNK = PADPOS // 128          # 34 key chunks (2 rows each)
NSUB = H // 2                   # 32 query subs (128 queries each)
SCALE = 1.0 / np.sqrt(np.float32(CH))
LAG = 5

# CB (bf16 const block) column layout
CB_W1T0 = 0
CB_W1T1 = 128
CB_W2T = 256
CB_WOT = 384          # (128, 256)
CB_MASK = 640         # (128, 384)
CB_TWOS = 1024        # (128, 128) of 2.0
CB_D512 = 1152        # row 0: (1, 512) pad-correction D
CB_BOUT1 = 1664       # row 0: (1, 128) bout[128:256]
CB_ONES512 = 1792     # row 0: (1, 512) of 1.0
CB_COLS = 2304


def _build_mask_and_D():
    """maskC: (128, 384) {0,1}; col 128*a+q is the score of key (chunk c,
    pos p) vs query q of sub s = c-2+a.  Valid iff |2-2a + p//64 - q//64|
    <= 2 and |p%64 - q%64| <= 2.   D: (512,) = 5*cnt(qx) tiled (the number
    of window slots per query that fall off the row ends; each contributes
    exp(0)=1 to the reference softmax denominator)."""
    m = np.zeros((128, 384), dtype=np.float32)
    for a in range(3):
        for p in range(128):
            for q in range(128):
                dy = 2 - 2 * a + p // 64 - q // 64
                if abs(dy) <= RAD and abs(p % 64 - q % 64) <= RAD:
                    m[p, 128 * a + q] = 1.0
    cnt = np.array([sum(1 for dx in range(-RAD, RAD + 1) if not 0 <= qx + dx < W)
                    for qx in range(W)], dtype=np.float32)
    Drow = 5.0 * np.concatenate([cnt, cnt, cnt, cnt])   # (256,) -> tile to 512
    D512 = np.concatenate([Drow, Drow])[:512]
    return m, D512


def build_nc(repeat=1, sim_safe=False, dbg=False):
    nc = bacc.Bacc("TRN2", target_bir_lowering=False, debug=False, num_devices=8)

    x_d = nc.declare_dram_parameter("x", [CIN, HW], BF16, isOutput=False)
    cb_d = nc.declare_dram_parameter("CB", [128, CB_COLS], BF16, isOutput=False)
    bf_d = nc.declare_dram_parameter("BF", [128, 4], F32, isOutput=False)
    out_d = nc.declare_dram_parameter("out", [CIN, HW], BF16, isOutput=True)
    if dbg:
        dbg_d = {nm: nc.declare_dram_parameter(f"dbg_{nm}", shp, dt, isOutput=True)
                 for nm, shp, dt in [
                     ("hpad", [128, PADPOS], BF16), ("hT", [128, PADPOS], BF16),
                     ("attnm", [128, NCHUNK * 384], BF16), ("Pg", [128, HW], BF16),
                     ("rden", [128, HW], F32), ("spg", [128, HW], BF16)]}

    with tile.TileContext(nc) as tc:
        with (
            tc.tile_pool(name="per", bufs=1) as per,
            tc.tile_pool(name="psc", bufs=3, space="PSUM") as psc,   # scores
            tc.tile_pool(name="psa", bufs=2, space="PSUM") as psa,   # conv2 / sample
            tc.tile_pool(name="pdn", bufs=2, space="PSUM") as pdn,   # conv1 / den
            tc.tile_pool(name="pou", bufs=1, space="PSUM") as pou,   # convout
        ):
            xsb0 = per.tile([128, HW], BF16, tag="xsb0")
            xsb1 = per.tile([128, HW], BF16, tag="xsb1")
            hpad = per.tile([128, PADPOS], BF16, tag="hpad")
            hT = per.tile([128, PADPOS], BF16, tag="hT")
            attnm = per.tile([128, NCHUNK * 384], BF16, tag="attnm")
            Pg = per.tile([128, HW], BF16, tag="Pg")
            spg = per.tile([128, HW], BF16, tag="spg")
            outsb = per.tile([128, 2 * HW], BF16, tag="outsb")
            rden = per.tile([128, HW], F32, tag="rden")
            cb = per.tile([128, CB_COLS], BF16, tag="cb")
            bfc = per.tile([128, 4], F32, tag="bfc")

            for _rep in range(repeat):
                # ---- input + const DMAs (4KB-contiguous runs, three queues).
                # Ordering: tiny BF first (fuels the PE warmup by ~9us), then
                # conv1/conv2 weights, then x pieces on separate queues, then
                # the big remaining consts (masks etc., not needed until the
                # chunk pipeline).
                nc.sync.dma_start(xsb0[:, 0:2048], x_d[0:128, 0:2048])
                nc.scalar.dma_start(bfc[:], bf_d[:])
                nc.scalar.dma_start(cb[:, 0:384], cb_d[:, 0:384])
                nc.scalar.dma_start(xsb1[:, 0:2048], x_d[128:256, 0:2048])
                nc.sync.dma_start(xsb0[:, 2048:4096], x_d[0:128, 2048:4096])
                nc.gpsimd.dma_start(xsb1[:, 2048:4096], x_d[128:256, 2048:4096])
                nc.scalar.dma_start(cb[:, 384:CB_COLS], cb_d[:, 384:CB_COLS])
                nc.gpsimd.memset(hpad[:, 0:128], 0.0)
                nc.gpsimd.memset(hpad[:, PADPOS - 128:PADPOS], 0.0)

                # ---- PE warmup: tiny matmuls on the early-landing BF tile
                # keep the PE continuously busy through the x-DMA wait so the
                # p-state ramps to full clock (needs ~3us continuous) before
                # conv1 starts
                warm = pou.tile([128, 512], F32, tag="po", name="warm")
                for _w in range(30):
                    nc.tensor.matmul(warm[0:4, 0:4], bfc[:, 0:4], bfc[:, 0:4],
                                     start=True, stop=True)

                # ---- P1: conv1 (+bias on DVE), conv2 gate lagging 2 tiles
                def emit_conv1(t):
                    pc = pdn.tile([128, 512], F32, tag="pc")
                    sl = slice(512 * t, 512 * (t + 1))
                    nc.tensor.matmul(pc[:], cb[:, CB_W1T0:CB_W1T0 + 128],
                                     xsb0[:, sl], start=True, stop=False)
                    nc.tensor.matmul(pc[:], cb[:, CB_W1T1:CB_W1T1 + 128],
                                     xsb1[:, sl], start=False, stop=True)
                    nc.vector.tensor_scalar(
                        out=hpad[:, 128 + 512 * t:128 + 512 * (t + 1)], in0=pc[:],
                        scalar1=bfc[:, 0:1], scalar2=None, op0=ALU.add)

                def emit_conv2(t):
                    pz = psa.tile([128, 512], F32, tag="pz")
                    hsl = slice(128 + 512 * t, 128 + 512 * (t + 1))
                    sl = slice(512 * t, 512 * (t + 1))
                    nc.tensor.matmul(pz[:], cb[:, CB_W2T:CB_W2T + 128],
                                     hpad[:, hsl], start=True, stop=True)
                    # Pg = 1 + relu(tanh(z/2 + b2/2)) = 2*sigmoid(relu(z))
                    nc.scalar.activation(Pg[:, sl], pz[:], AF.Tanh,
                                         scale=0.5, bias=bfc[:, 1:2])
                    nc.vector.tensor_scalar(out=Pg[:, sl], in0=Pg[:, sl],
                                            scalar1=0.0, scalar2=1.0,
                                            op0=ALU.max, op1=ALU.add)

                for t in range(8):
                    emit_conv1(t)
                    if t >= 2:
                        emit_conv2(t - 2)
                emit_conv2(6)
                emit_conv2(7)

                # ---- hT via DMA transpose (4 quarters of 9/9/9/7 chunks)
                for q in range(4):
                    c0, c1 = 9 * q, min(NCHUNK, 9 * (q + 1))
                    cols = slice(128 * c0, 128 * c1)
                    nc.sync.dma_start(
                        hT[:, cols].rearrange("p (c k) -> p c k", k=128),
                        hpad[:, cols], transpose=True)

                # ---- P2: per-chunk pipeline
                sc_meta = {}
                samp_tiles = {}
                den_tiles = {}
                po_tiles = {}

                def emit_score(c):
                    lo, hi = max(0, c - 2), min(NSUB - 1, c)
                    alo, ahi = lo - (c - 2), hi - (c - 2)
                    sc = psc.tile([128, 512], F32, tag="sc")
                    sc_meta[c] = (alo, ahi)
                    psl = slice(128 * alo, 128 * (ahi + 1))
                    nc.tensor.matmul(sc[:, psl],
                                     hpad[:, 128 * c:128 * (c + 1)],
                                     hpad[:, 128 * (lo + 1):128 * (hi + 2)],
                                     start=True, stop=True)
                    asl = attnm[:, 384 * c + 128 * alo:384 * c + 128 * (ahi + 1)]
                    nc.scalar.activation(asl, sc[:, psl], AF.Exp, scale=float(SCALE))
                    meng = nc.vector if c % 3 == 0 else nc.gpsimd
                    meng.tensor_tensor(
                        out=asl, in0=asl,
                        in1=cb[:, CB_MASK + 128 * alo:CB_MASK + 128 * (ahi + 1)],
                        op=ALU.mult)

                def emit_sampden(c):
                    alo, ahi = sc_meta[c]
                    if c % 4 == 0 and c < 32:
                        # tiles for supersub G=c//4: first write is chunk 4G.
                        # Each bank gets exactly ONE start=True (the K=1
                        # preload); all later MMs accumulate, so no open
                        # accumulation group is ever wiped.
                        G = c // 4
                        den_tiles[G] = pdn.tile([128, 512], F32, tag="pc",
                                                name=f"dn{G}")
                        nc.tensor.matmul(den_tiles[G][:],
                                         cb[0:1, CB_TWOS:CB_TWOS + 128],
                                         cb[0:1, CB_D512:CB_D512 + 512],
                                         start=True, stop=False,
                                         skip_group_check=True)
                        samp_tiles[G] = psa.tile([128, 512], F32, tag="pz",
                                                 name=f"sp{G}")
                        nc.tensor.matmul(samp_tiles[G][:],
                                         cb[32:33, CB_D512:CB_D512 + 128],
                                         cb[32:33, CB_D512:CB_D512 + 512],
                                         start=True, stop=False,
                                         skip_group_check=True)
                    # sample + den: one MM per contiguous run of subs in the
                    # same psum tile (chunk c's 3 blocks share lhsT hT[c] and
                    # are contiguous in attnm)
                    a = alo
                    while a <= ahi:
                        s = c - 2 + a
                        G, j = s // 4, s % 4
                        na = min(ahi - a + 1, 4 - j)
                        blk = attnm[:, 384 * c + 128 * a:384 * c + 128 * (a + na)]
                        nc.tensor.matmul(
                            samp_tiles[G][:, 128 * j:128 * (j + na)],
                            hT[:, 128 * c:128 * (c + 1)], blk,
                            start=False, stop=(alo == 0), skip_group_check=True)
                        nc.tensor.matmul(
                            den_tiles[G][:, 128 * j:128 * (j + na)],
                            cb[:, CB_TWOS:CB_TWOS + 128], blk,
                            start=False, stop=(alo == 0), skip_group_check=True)
                        a += na

                def emit_finish1(G):
                    # softmax-normalize + gate on DVE (2 chunks before the
                    # convout MMs enter the PE queue, so the PE never stalls
                    # on this chain)
                    qsl = slice(512 * G, 512 * (G + 1))
                    sp_ = samp_tiles.pop(G)
                    den_ = den_tiles.pop(G)
                    nc.vector.reciprocal_approx_fast(out=rden[:, qsl], in_=den_[:])
                    nc.vector.tensor_tensor(out=spg[:, qsl], in0=sp_[:],
                                            in1=Pg[:, qsl], op=ALU.mult)
                    nc.vector.tensor_tensor(out=spg[:, qsl], in0=spg[:, qsl],
                                            in1=rden[:, qsl], op=ALU.mult)

                def emit_finish2(G):
                    qsl = slice(512 * G, 512 * (G + 1))
                    po = pou.tile([128, 512], F32, tag="po", name=f"po{G}a")
                    po_tiles[G] = po
                    nc.tensor.matmul(po[:], cb[:, CB_WOT:CB_WOT + 128],
                                     spg[:, qsl], start=True, stop=True)
                    nc.vector.tensor_scalar(
                        out=outsb[:, qsl], in0=po[:],
                        scalar1=bfc[:, 2:3], scalar2=None, op0=ALU.add)
                    if G % 2 == 1:
                        # oc0 rows for supersub pair (G-1, G): 2KB runs
                        psl = slice(512 * (G - 1), 512 * (G + 1))
                        nc.sync.dma_start(out_d[0:128, psl], outsb[:, psl])

                def emit_finish3(G):
                    qsl = slice(512 * G, 512 * (G + 1))
                    po_tiles.pop(G)
                    po2 = pou.tile([128, 512], F32, tag="po", name=f"po{G}b")
                    nc.tensor.matmul(po2[:], cb[0:1, CB_BOUT1:CB_BOUT1 + 128],
                                     cb[0:1, CB_ONES512:CB_ONES512 + 512],
                                     start=True, stop=False, skip_group_check=True)
                    nc.tensor.matmul(po2[:], cb[:, CB_WOT + 128:CB_WOT + 256],
                                     spg[:, qsl], start=False, stop=True,
                                     skip_group_check=True)
                    nc.scalar.activation(outsb[:, 4096 + 512 * G:4096 + 512 * (G + 1)],
                                         po2[:], AF.Copy)
                    if G % 2 == 1:
                        psl = slice(512 * (G - 1), 512 * (G + 1))
                        nc.scalar.dma_start(out_d[128:256, psl],
                                            outsb[:, 4096 + psl.start:4096 + psl.stop])

                for cc in range(NCHUNK + LAG + 4):
                    if cc < NCHUNK:
                        emit_score(cc)
                    d = cc - LAG
                    if 0 <= d < NCHUNK:
                        emit_sampden(d)
                    if d >= 5 and (d - 5) % 4 == 0 and (d - 5) // 4 < 8:
                        emit_finish1((d - 5) // 4)
                    if d >= 7 and (d - 7) % 4 == 0 and (d - 7) // 4 < 8:
                        emit_finish2((d - 7) // 4)
                    if d >= 8 and (d - 8) % 4 == 0 and (d - 8) // 4 < 8:
                        emit_finish3((d - 8) // 4)

                if dbg:
                    for nm, t in [("hpad", hpad), ("hT", hT), ("attnm", attnm),
                                  ("Pg", Pg), ("rden", rden), ("spg", spg)]:
                        nc.sync.dma_start(dbg_d[nm][:], t[:])

    return nc


def _prep_inputs(x, W1, b1, W2, b2, Wout, bout):
    maskC, D512 = _build_mask_and_D()
    bf = ml_dtypes.bfloat16
    CB = np.zeros((128, CB_COLS), np.float32)
    W1T = np.ascontiguousarray(np.asarray(W1, np.float32).T)   # (256, 128)
    CB[:, CB_W1T0:CB_W1T0 + 128] = W1T[0:128]
    CB[:, CB_W1T1:CB_W1T1 + 128] = W1T[128:256]
    CB[:, CB_W2T:CB_W2T + 128] = np.asarray(W2, np.float32).T
    CB[:, CB_WOT:CB_WOT + 256] = np.asarray(Wout, np.float32).T
    CB[:, CB_MASK:CB_MASK + 384] = maskC
    CB[:, CB_TWOS:CB_TWOS + 128] = 2.0
    CB[0, CB_D512:CB_D512 + 512] = D512
    CB[0, CB_BOUT1:CB_BOUT1 + 128] = np.asarray(bout, np.float32)[128:256]
    CB[0, CB_ONES512:CB_ONES512 + 512] = 1.0
    BF = np.zeros((128, 4), np.float32)
    BF[:, 0] = np.asarray(b1, np.float32)
    BF[:, 1] = 0.5 * np.asarray(b2, np.float32)
    BF[:, 2] = np.asarray(bout, np.float32)[0:128]
    common = {"CB": CB.astype(bf), "BF": BF}
    in_maps = []
    for i in range(N):
        m = dict(common)
        m["x"] = np.ascontiguousarray(
            np.asarray(x[i], np.float32).reshape(CIN, HW)).astype(bf)
        in_maps.append(m)
    return in_maps


_CACHED = {}


def kernel(x, W1, b1, W2, b2, Wout, bout):
    if "nc" not in _CACHED:
        nc = build_nc()
        nc.finalize()
        _CACHED["nc"] = nc
    nc = _CACHED["nc"]
    in_maps = _prep_inputs(x, W1, b1, W2, b2, Wout, bout)
    res = run_bass_kernel_spmd(nc, in_maps, core_ids=list(range(N)))
    out = np.stack([np.asarray(res.results[i]["out"], dtype=np.float32)
                    .reshape(CIN, H, W) for i in range(N)])
    return out
